# revision 5
# baseline (speedup 1.0000x reference)
"""Multi-head attention (N=2, S=2048, D=1024, H=16) on 8 TRN2 NeuronCores.

Sharding: core c handles batch b = c//4 and head group g = c%4 (4 heads).
Column-parallel qkv projection, per-head attention, row-parallel o_proj;
the 4 partial o_proj outputs per batch are summed on the host (unshard).

Per-core device kernel (all fp32 storage, float32r matmuls):
  phase 1: qkT = wqkT.T @ xT   (q/k transposed layouts, head pairs stacked
           on partitions), v = xT.T @ wvT (natural layout, with a ones
           column appended per head for fused softmax-denominator)
  phase 2: per head pair / sq-block: scoresT = kT.T @ qT -> exp (ACT,
           fused 1/sqrt(hd) scale) -> valuesT(+denom) = v_ext.T @ attnT
  phase 3: divide by denom (reciprocal + PE ones-broadcast + DVE mul)
  phase 4: o_part = valuesT.T @ owT -> DMA out
"""

import numpy as np

import concourse.bass as bass  # noqa: F401
import concourse.mybir as mybir
import concourse.tile as tile
from concourse import bacc
from concourse.bass_utils import run_bass_kernel_spmd

f32 = mybir.dt.float32
f32r = mybir.dt.float32r
AF = mybir.ActivationFunctionType

P = 128
N, S, D, H = 2, 2048, 1024, 16
HD = D // H                    # 64
NH = 4                         # heads per core
SCALE = float(1.0 / np.sqrt(np.float32(HD)))
E_QK = 2 * NH * HD             # 512 qk rows per core
E_V = NH * HD                  # 256
DL = NH * HD                   # 256 local d for o_proj
SQB = 512                      # sq block
NSQB = S // SQB                # 4
SKT = S // P                   # 16 sk tiles

# tuning knobs
SKG = 1                        # sk tiles per exp group
SC_BUFS = 2                    # score psum double buffering
ATTN_BUFS = 4


def _emit_body(nc, tc, t, rep):
    from contextlib import ExitStack

    with ExitStack() as ctx:
        const = ctx.enter_context(tc.tile_pool(name=f"const{rep}", bufs=1))
        persist = ctx.enter_context(tc.tile_pool(name=f"persist{rep}", bufs=1))

        wqk_sb = const.tile([P, 8, E_QK], f32r, name="wqk_sb")
        nc.sync.dma_start(wqk_sb[:], t["wqkT"].rearrange("(a p) e -> p a e", p=P))
        wv_sb = const.tile([P, 8, E_V], f32r, name="wv_sb")
        nc.sync.dma_start(wv_sb[:], t["wvT"].rearrange("(a p) e -> p a e", p=P))
        ow_sb = const.tile([P, 2, D], f32r, name="ow_sb")
        nc.sync.dma_start(ow_sb[:], t["owT"].rearrange("(a p) e -> p a e", p=P))
        ones_sb = const.tile([65, HD], f32r, name="ones_sb")
        nc.sync.dma_start(ones_sb[64:65, :], t["onesd"][64:65, 0:HD])

        qT = persist.tile([P, 2, S], f32r, name="qT")
        kT = persist.tile([P, 2, S], f32r, name="kT")
        v_sb = persist.tile([P, SKT, NH * 65], f32r, name="v_sb")
        vals = persist.tile([P, 2, S], f32r, name="vals")
        nc.sync.dma_start(
            v_sb.rearrange("p a (h e) -> p a h e", e=65)[:, :, :, 64:65],
            t["onesd"].rearrange("p (a h) -> p a h", h=NH)[:, :, :, None],
        )

        # ---------------- phase 1: projections ----------------
        with (
            tc.tile_pool(name=f"xp{rep}", bufs=2) as xp,
            tc.tile_pool(name=f"ps1{rep}", bufs=3, space="PSUM") as ps1,
        ):
            xT_r = t["xT"].rearrange("(a p) s -> p a s", p=P)
            for sb in range(4):
                ss = slice(sb * 512, (sb + 1) * 512)
                xt = xp.tile([P, 8, 512], f32r, name="xt")
                nc.sync.dma_start(xt[:], xT_r[:, :, ss])
                for et in range(4):
                    pq = ps1.tile([P, 512], f32, name="pq", tag="pq")
                    for a in range(8):
                        nc.tensor.matmul(
                            pq[:],
                            wqk_sb[:, a, et * 128:(et + 1) * 128],
                            xt[:, a, :],
                            start=(a == 0),
                            stop=(a == 7),
                        )
                    dst = (qT if et % 2 == 0 else kT)[:, et // 2, ss]
                    nc.any.tensor_copy(dst, pq[:])
                for st in range(4):
                    pv = ps1.tile([P, E_V], f32, name="pv", tag="pv")
                    for a in range(8):
                        nc.tensor.matmul(
                            pv[:],
                            xt[:, a, st * 128:(st + 1) * 128],
                            wv_sb[:, a, :],
                            start=(a == 0),
                            stop=(a == 7),
                        )
                    so = sb * 4 + st
                    nc.any.tensor_copy(
                        v_sb[:, so].rearrange("p (h e) -> p h e", e=65)[:, :, 0:64],
                        pv.rearrange("p (h e) -> p h e", e=64),
                    )

        # ---------------- phase 2-4: attention + o_proj ----------------
        with (
            tc.tile_pool(name=f"scp{rep}", bufs=SC_BUFS, space="PSUM") as scp,
            tc.tile_pool(name=f"vp{rep}", bufs=3, space="PSUM") as vp,
            tc.tile_pool(name=f"mp{rep}", bufs=1, space="PSUM") as mp,
            tc.tile_pool(name=f"attn{rep}", bufs=ATTN_BUFS) as attnp,
            tc.tile_pool(name=f"sm{rep}", bufs=2) as sm,
            tc.tile_pool(name=f"outp{rep}", bufs=3) as outp,
        ):
            for qb in range(NSQB):
                sqs = slice(qb * SQB, (qb + 1) * SQB)
                for pr in range(2):
                    vps = [
                        vp.tile([65, SQB], f32, name=f"vps{h}", tag="vps")
                        for h in range(2)
                    ]
                    for g in range(SKT // SKG):
                        sc = scp.tile([P, SKG, 2, SQB], f32, name="sc", tag="sc")
                        at = attnp.tile([P, SKG, 2, SQB], f32r, name="at", tag="at")
                        for j in range(SKG):
                            sk = g * SKG + j
                            for h in range(2):
                                nc.tensor.matmul(
                                    sc[:, j, h, :],
                                    kT[h * 64:(h + 1) * 64, pr,
                                       sk * 128:(sk + 1) * 128],
                                    qT[h * 64:(h + 1) * 64, pr, sqs],
                                    start=True,
                                    stop=True,
                                )
                        nc.scalar.activation(at[:], sc[:], AF.Exp, scale=SCALE)
                        for j in range(SKG):
                            sk = g * SKG + j
                            for h in range(2):
                                lh = pr * 2 + h
                                nc.tensor.matmul(
                                    vps[h][:],
                                    v_sb[:, sk, lh * 65:(lh + 1) * 65],
                                    at[:, j, h, :],
                                    start=(sk == 0),
                                    stop=(sk == SKT - 1),
                                )
                    for h in range(2):
                        recr = sm.tile([65, SQB], f32r, name="recr", tag="recr")
                        with nc.allow_low_precision(reason="tf32 softmax recip"):
                            nc.vector.reciprocal(recr[64:65, :], vps[h][64:65, :])
                        bc = mp.tile([128, SQB], f32, name="bc", tag="m")[0:64, :]
                        nc.tensor.matmul(
                            bc[:],
                            ones_sb[64:65, :],
                            recr[64:65, :],
                            start=True,
                            stop=True,
                        )
                        bcs = sm.tile([64, SQB], f32, name="bcs", tag="bcs")
                        nc.vector.tensor_copy(bcs[:], bc[:])
                        if h == 0:
                            nc.vector.tensor_mul(
                                out=vals[0:64, pr, sqs],
                                in0=vps[h][0:64, :],
                                in1=bcs[:],
                            )
                        else:
                            tmp = sm.tile([64, SQB], f32r, name="tmpv", tag="tmpv")
                            nc.vector.tensor_mul(
                                out=tmp[:], in0=vps[h][0:64, :], in1=bcs[:]
                            )
                            nc.sync.dma_start(vals[64:128, pr, sqs], tmp[:])
                # o_proj for the 4 s-tiles of this q block
                for st in range(4):
                    s0 = qb * 4 + st
                    for eb in range(2):
                        ops = mp.tile([P, 512], f32, name="ops", tag="m")
                        for a in range(2):
                            nc.tensor.matmul(
                                ops[:],
                                vals[:, a, s0 * 128:(s0 + 1) * 128],
                                ow_sb[:, a, eb * 512:(eb + 1) * 512],
                                start=(a == 0),
                                stop=(a == 1),
                            )
                        ot = outp.tile([P, 512], f32, name="ot")
                        nc.vector.tensor_copy(ot[:], ops[:])
                        nc.sync.dma_start(
                            t["o"][s0 * 128:(s0 + 1) * 128,
                                   eb * 512:(eb + 1) * 512],
                            ot[:],
                        )


def build_nc(repeats: int = 1):
    nc = bacc.Bacc(None, target_bir_lowering=False)
    t = {
        "xT": nc.dram_tensor("xT", [D, S], f32r, kind="ExternalInput")[:, :],
        "wqkT": nc.dram_tensor("wqkT", [D, E_QK], f32r, kind="ExternalInput")[:, :],
        "wvT": nc.dram_tensor("wvT", [D, E_V], f32r, kind="ExternalInput")[:, :],
        "owT": nc.dram_tensor("owT", [DL, D], f32r, kind="ExternalInput")[:, :],
        "onesd": nc.dram_tensor("onesd", [P, SKT * NH], f32r,
                                kind="ExternalInput")[:, :],
        "o": nc.dram_tensor("o", [S, D], f32, kind="ExternalOutput")[:, :],
    }
    with tile.TileContext(nc) as tc:
        for rep in range(repeats):
            _emit_body(nc, tc, t, rep)
    nc.compile()
    return nc


def tf32_round(a):
    u = np.ascontiguousarray(a, dtype=np.float32).view(np.uint32)
    r = (u + np.uint32(0xFFF) + ((u >> np.uint32(13)) & np.uint32(1))) & ~np.uint32(
        0x1FFF
    )
    return r.view(np.float32)


def make_in_maps(x, qkv_w, o_w):
    x = np.ascontiguousarray(np.asarray(x, dtype=np.float32))
    qkv_w = np.ascontiguousarray(np.asarray(qkv_w, dtype=np.float32))
    o_w = np.ascontiguousarray(np.asarray(o_w, dtype=np.float32))
    in_maps = []
    for c in range(8):
        b, g = c // 4, c % 4
        heads = [4 * g + i for i in range(NH)]
        xT = np.ascontiguousarray(x[b].T)
        wq = [qkv_w[h * 192:h * 192 + 64] for h in heads]
        wk = [qkv_w[h * 192 + 64:h * 192 + 128] for h in heads]
        wv = [qkv_w[h * 192 + 128:h * 192 + 192] for h in heads]
        wqk = np.concatenate(
            [wq[0], wq[1], wk[0], wk[1], wq[2], wq[3], wk[2], wk[3]], axis=0
        )
        wqkT = np.ascontiguousarray(wqk.T)
        wvT = np.ascontiguousarray(np.concatenate(wv, axis=0).T)
        cols = np.concatenate([np.arange(h * 64, h * 64 + 64) for h in heads])
        owT = np.ascontiguousarray(o_w[:, cols].T)
        in_maps.append({"xT": tf32_round(xT), "wqkT": tf32_round(wqkT),
                        "wvT": tf32_round(wvT), "owT": tf32_round(owT),
                        "onesd": np.ones((P, SKT * NH), np.float32)})
    return in_maps


_NC_CACHE = {}


def _get_nc(repeats=1):
    if repeats not in _NC_CACHE:
        _NC_CACHE[repeats] = build_nc(repeats)
    return _NC_CACHE[repeats]


def run_on_hw(x, qkv_w, o_w, repeats=1, **kwargs):
    nc = _get_nc(repeats)
    in_maps = make_in_maps(x, qkv_w, o_w)
    res = run_bass_kernel_spmd(nc, in_maps, core_ids=list(range(8)), **kwargs)
    out = np.zeros((N, S, D), dtype=np.float32)
    for c in range(8):
        out[c // 4] += res.results[c]["o"]
    return out, res


def kernel(x, qkv_w, o_w):
    out, _ = run_on_hw(x, qkv_w, o_w)
    return out


# revision 6
# speedup vs baseline: 1.2161x; 1.2161x over previous
"""Multi-head attention (N=2, S=2048, D=1024, H=16) on 8 TRN2 NeuronCores.

Sharding: core c handles batch b = c//4 and head group g = c%4 (4 heads).
Column-parallel qkv projection, per-head attention, row-parallel o_proj;
the 4 partial o_proj outputs per batch are summed on the host (unshard).

Per-core device kernel (all fp32 storage, float32r matmuls):
  phase 1: qkT = wqkT.T @ xT   (q/k transposed layouts, head pairs stacked
           on partitions), v = xT.T @ wvT (natural layout, with a ones
           column appended per head for fused softmax-denominator)
  phase 2: per head pair / sq-block: scoresT = kT.T @ qT -> exp (ACT,
           fused 1/sqrt(hd) scale) -> valuesT(+denom) = v_ext.T @ attnT
  phase 3: divide by denom (reciprocal + PE ones-broadcast + DVE mul)
  phase 4: o_part = valuesT.T @ owT -> DMA out
"""

import numpy as np

import concourse.bass as bass  # noqa: F401
import concourse.mybir as mybir
import concourse.tile as tile
from concourse import bacc
from concourse.bass_utils import run_bass_kernel_spmd

f32 = mybir.dt.float32
f32r = mybir.dt.float32r
AF = mybir.ActivationFunctionType

P = 128
N, S, D, H = 2, 2048, 1024, 16
HD = D // H                    # 64
NH = 4                         # heads per core
SCALE = float(1.0 / np.sqrt(np.float32(HD)))
E_QK = 2 * NH * HD             # 512 qk rows per core
E_V = NH * HD                  # 256
DL = NH * HD                   # 256 local d for o_proj
SQB = 512                      # sq block
NSQB = S // SQB                # 4
SKT = S // P                   # 16 sk tiles

# tuning knobs
SKG = 1                        # sk tiles per exp group
SC_BUFS = 2                    # score psum double buffering
ATTN_BUFS = 4


def _emit_body(nc, tc, t, rep):
    from contextlib import ExitStack

    with ExitStack() as ctx:
        const = ctx.enter_context(tc.tile_pool(name=f"const{rep}", bufs=1))
        persist = ctx.enter_context(tc.tile_pool(name=f"persist{rep}", bufs=1))

        wqk_sb = const.tile([P, 8, E_QK], f32r, name="wqk_sb")
        nc.sync.dma_start(wqk_sb[:], t["wqkT"].rearrange("(a p) e -> p a e", p=P))
        wv_sb = const.tile([P, 8, E_V], f32r, name="wv_sb")
        nc.sync.dma_start(wv_sb[:], t["wvT"].rearrange("(a p) e -> p a e", p=P))
        ow_sb = const.tile([P, 2, D], f32r, name="ow_sb")
        nc.sync.dma_start(ow_sb[:], t["owT"].rearrange("(a p) e -> p a e", p=P))
        ones_sb = const.tile([97, HD], f32r, name="ones_sb")
        nc.sync.dma_start(ones_sb[32:33, :], t["onesd"][32:33, 0:HD])
        nc.sync.dma_start(ones_sb[96:97, :], t["onesd"][96:97, 0:HD])
        ones_col = const.tile([P, 1], f32r, name="ones_col")
        nc.sync.dma_start(ones_col[:, :], t["onesd"][:, 0:1])

        qT = persist.tile([P, 2, S], f32r, name="qT")
        kT = persist.tile([P, 2, S], f32r, name="kT")
        v_sb = persist.tile([P, SKT, NH * HD], f32r, name="v_sb")
        vals = persist.tile([P, 2, S], f32r, name="vals")

        # ---------------- phase 1: projections ----------------
        with (
            tc.tile_pool(name=f"xp{rep}", bufs=2) as xp,
            tc.tile_pool(name=f"ps1{rep}", bufs=3, space="PSUM") as ps1,
        ):
            xT_r = t["xT"].rearrange("(a p) s -> p a s", p=P)
            for sb in range(4):
                ss = slice(sb * 512, (sb + 1) * 512)
                xt = xp.tile([P, 8, 512], f32r, name="xt")
                nc.sync.dma_start(xt[:], xT_r[:, :, ss])
                for et in range(4):
                    pq = ps1.tile([P, 512], f32, name="pq", tag="pq")
                    for a in range(8):
                        nc.tensor.matmul(
                            pq[:],
                            wqk_sb[:, a, et * 128:(et + 1) * 128],
                            xt[:, a, :],
                            start=(a == 0),
                            stop=(a == 7),
                        )
                    dst = (qT if et % 2 == 0 else kT)[:, et // 2, ss]
                    nc.any.tensor_copy(dst, pq[:])
                for st in range(4):
                    pv = ps1.tile([P, E_V], f32, name="pv", tag="pv")
                    for a in range(8):
                        nc.tensor.matmul(
                            pv[:],
                            xt[:, a, st * 128:(st + 1) * 128],
                            wv_sb[:, a, :],
                            start=(a == 0),
                            stop=(a == 7),
                        )
                    so = sb * 4 + st
                    nc.any.tensor_copy(v_sb[:, so], pv[:])

        # ---------------- phase 2-4: attention + o_proj ----------------
        with (
            tc.tile_pool(name=f"scp{rep}", bufs=SC_BUFS, space="PSUM") as scp,
            tc.tile_pool(name=f"vp{rep}", bufs=2, space="PSUM") as vp,
            tc.tile_pool(name=f"dp{rep}", bufs=1, space="PSUM") as dp,
            tc.tile_pool(name=f"mp{rep}", bufs=1, space="PSUM") as mp,
            tc.tile_pool(name=f"attn{rep}", bufs=ATTN_BUFS) as attnp,
            tc.tile_pool(name=f"sm{rep}", bufs=2) as sm,
            tc.tile_pool(name=f"outp{rep}", bufs=3) as outp,
        ):
            for qb in range(NSQB):
                sqs = slice(qb * SQB, (qb + 1) * SQB)
                for pr in range(2):
                    vps = vp.tile([P, SQB], f32, name="vps", tag="vps")
                    dps = dp.tile([P, SQB], f32, name="dps", tag="dps")
                    for g in range(SKT // SKG):
                        sc = scp.tile([P, SKG, 2, SQB], f32, name="sc", tag="sc")
                        at = attnp.tile([P, SKG, 2, SQB], f32r, name="at", tag="at")
                        for j in range(SKG):
                            sk = g * SKG + j
                            for h in range(2):
                                nc.tensor.matmul(
                                    sc[:, j, h, :],
                                    kT[h * 64:(h + 1) * 64, pr,
                                       sk * 128:(sk + 1) * 128],
                                    qT[h * 64:(h + 1) * 64, pr, sqs],
                                    start=True,
                                    stop=True,
                                )
                        nc.scalar.activation(at[:], sc[:], AF.Exp, scale=SCALE)
                        for j in range(SKG):
                            sk = g * SKG + j
                            # column-packed pair: h0 -> cols 0-63, h1 -> 64-127
                            for h in range(2):
                                lh = pr * 2 + h
                                nc.tensor.matmul(
                                    vps[h * 64:(h + 1) * 64, :],
                                    v_sb[:, sk, lh * HD:(lh + 1) * HD],
                                    at[:, j, h, :],
                                    start=(sk == 0),
                                    stop=(sk == SKT - 1),
                                )
                            # packed denominator rows at col positions 32 / 96
                            for h in range(2):
                                nc.tensor.matmul(
                                    dps[32 + h * 64:33 + h * 64, :],
                                    ones_col[:, :],
                                    at[:, j, h, :],
                                    start=(sk == 0),
                                    stop=(sk == SKT - 1),
                                )
                    for h in range(2):
                        hb = h * 64
                        recr = sm.tile([97, SQB], f32r, name="recr", tag="recr")
                        with nc.allow_low_precision(reason="tf32 softmax recip"):
                            nc.vector.reciprocal(
                                recr[32 + hb:33 + hb, :], dps[32 + hb:33 + hb, :]
                            )
                        bc = mp.tile([P, SQB], f32, name="bc", tag="m")[hb:hb + 64, :]
                        nc.tensor.matmul(
                            bc[:],
                            ones_sb[32 + hb:33 + hb, :],
                            recr[32 + hb:33 + hb, :],
                            start=True,
                            stop=True,
                        )
                        bcs = sm.tile([P, SQB], f32, name="bcs", tag="bcs")
                        nc.vector.tensor_copy(bcs[hb:hb + 64, :], bc[:])
                        nc.vector.tensor_mul(
                            out=vals[hb:hb + 64, pr, sqs],
                            in0=vps[hb:hb + 64, :],
                            in1=bcs[hb:hb + 64, :],
                        )
                # o_proj for the 4 s-tiles of this q block
                for st in range(4):
                    s0 = qb * 4 + st
                    for eb in range(2):
                        ops = mp.tile([P, 512], f32, name="ops", tag="m")
                        for a in range(2):
                            nc.tensor.matmul(
                                ops[:],
                                vals[:, a, s0 * 128:(s0 + 1) * 128],
                                ow_sb[:, a, eb * 512:(eb + 1) * 512],
                                start=(a == 0),
                                stop=(a == 1),
                            )
                        ot = outp.tile([P, 512], f32, name="ot")
                        nc.vector.tensor_copy(ot[:], ops[:])
                        nc.sync.dma_start(
                            t["o"][s0 * 128:(s0 + 1) * 128,
                                   eb * 512:(eb + 1) * 512],
                            ot[:],
                        )


def build_nc(repeats: int = 1):
    nc = bacc.Bacc(None, target_bir_lowering=False)
    t = {
        "xT": nc.dram_tensor("xT", [D, S], f32r, kind="ExternalInput")[:, :],
        "wqkT": nc.dram_tensor("wqkT", [D, E_QK], f32r, kind="ExternalInput")[:, :],
        "wvT": nc.dram_tensor("wvT", [D, E_V], f32r, kind="ExternalInput")[:, :],
        "owT": nc.dram_tensor("owT", [DL, D], f32r, kind="ExternalInput")[:, :],
        "onesd": nc.dram_tensor("onesd", [P, SKT * NH], f32r,
                                kind="ExternalInput")[:, :],
        "o": nc.dram_tensor("o", [S, D], f32, kind="ExternalOutput")[:, :],
    }
    with tile.TileContext(nc) as tc:
        for rep in range(repeats):
            _emit_body(nc, tc, t, rep)
    nc.compile()
    return nc


def tf32_round(a):
    u = np.ascontiguousarray(a, dtype=np.float32).view(np.uint32)
    r = (u + np.uint32(0xFFF) + ((u >> np.uint32(13)) & np.uint32(1))) & ~np.uint32(
        0x1FFF
    )
    return r.view(np.float32)


def make_in_maps(x, qkv_w, o_w):
    x = np.ascontiguousarray(np.asarray(x, dtype=np.float32))
    qkv_w = np.ascontiguousarray(np.asarray(qkv_w, dtype=np.float32))
    o_w = np.ascontiguousarray(np.asarray(o_w, dtype=np.float32))
    in_maps = []
    for c in range(8):
        b, g = c // 4, c % 4
        heads = [4 * g + i for i in range(NH)]
        xT = np.ascontiguousarray(x[b].T)
        wq = [qkv_w[h * 192:h * 192 + 64] for h in heads]
        wk = [qkv_w[h * 192 + 64:h * 192 + 128] for h in heads]
        wv = [qkv_w[h * 192 + 128:h * 192 + 192] for h in heads]
        wqk = np.concatenate(
            [wq[0], wq[1], wk[0], wk[1], wq[2], wq[3], wk[2], wk[3]], axis=0
        )
        wqkT = np.ascontiguousarray(wqk.T)
        wvT = np.ascontiguousarray(np.concatenate(wv, axis=0).T)
        cols = np.concatenate([np.arange(h * 64, h * 64 + 64) for h in heads])
        owT = np.ascontiguousarray(o_w[:, cols].T)
        in_maps.append({"xT": tf32_round(xT), "wqkT": tf32_round(wqkT),
                        "wvT": tf32_round(wvT), "owT": tf32_round(owT),
                        "onesd": np.ones((P, SKT * NH), np.float32)})
    return in_maps


_NC_CACHE = {}


def _get_nc(repeats=1):
    if repeats not in _NC_CACHE:
        _NC_CACHE[repeats] = build_nc(repeats)
    return _NC_CACHE[repeats]


def run_on_hw(x, qkv_w, o_w, repeats=1, **kwargs):
    nc = _get_nc(repeats)
    in_maps = make_in_maps(x, qkv_w, o_w)
    res = run_bass_kernel_spmd(nc, in_maps, core_ids=list(range(8)), **kwargs)
    out = np.zeros((N, S, D), dtype=np.float32)
    for c in range(8):
        out[c // 4] += res.results[c]["o"]
    return out, res


def kernel(x, qkv_w, o_w):
    out, _ = run_on_hw(x, qkv_w, o_w)
    return out
